# revision 1
# baseline (speedup 1.0000x reference)
"""Multi-head self-attention Trainium2 kernel (Bass/Tile), 8-core SPMD.

Problem (hardcoded): B=2, S=2048, D_MODEL=1024, N_HEADS=16, HEAD_DIM=64,
mask == all-ones (no masking), dropout=0.

Sharding: core c handles batch b = c // 4 and head-quarter hq = c % 4
(heads 4*hq .. 4*hq+3).  QKV projections are column-parallel over the
head slice; attention is head-local; output projection is row-parallel
(each core produces a partial [S, D] output; host sums the 4 partials
per batch and adds bo + Wo @ bv).

Device layouts (per core):
  qT, kT: [256, 2048] as 2 sbuf tiles [128, 2048]  (partition = head dim,
          tile t holds heads 2t, 2t+1; scores matmuls use K=64 slices which
          auto-row-tile at positions 0/64)
  v_sb:   16 s-tiles [128, 4*65]; per head h cols h*65..h*65+63 = v,
          col h*65+64 = ones  ->  ctx matmul lhsT [128, 65] yields
          unnormalized ctx rows 0:64 and the softmax denominator in row 64.
  softmax: no max subtraction (scores ~ N(0,1), exp can't overflow);
          probs never normalized -- ctx is divided by the denominator.
  bv is NOT applied on device: softmax rows sum to 1, so +bv in V adds the
          constant row bv @ Wo_c.T to the output; host folds it with bo.
"""

import contextlib
import sys

sys.path.insert(0, "/opt/trn_rl_repo")

import numpy as np

import concourse.bacc as bacc
import concourse.tile as tile
from concourse import mybir
from concourse.bass_utils import run_bass_kernel_spmd

S = 2048
D = 1024
HPC = 4          # heads per core
DH = 64          # head dim
DC = HPC * DH    # 256 = projected dims per core
KC = D // 128    # 8 contraction chunks for projections
ST = S // 128    # 16 s-tiles
QC = S // 512    # 4 q-chunks
SCALE = DH ** -0.5

F32 = mybir.dt.float32
F32R = mybir.dt.float32r


def build_nc(use_f32r=True, exp_group=3, proj_own=False, sc_bufs=2, ctx_bufs=2, pj_bufs=2, probs_bufs=2, po_engine='vector', repeat=1, probe=(), fastboot=0):
    """Build the SPMD Bass program (same NEFF for all 8 cores)."""
    nc = bacc.Bacc(None, target_bir_lowering=False, debug=False, num_devices=8)
    MD = F32R if use_f32r else F32  # dtype for matmul operands

    xT = nc.dram_tensor("xT", [D, S], MD, kind="ExternalInput")
    wqT = nc.dram_tensor("wqT", [D, DC], MD, kind="ExternalInput")
    wkT = nc.dram_tensor("wkT", [D, DC], MD, kind="ExternalInput")
    wvT = nc.dram_tensor("wvT", [D, DC], MD, kind="ExternalInput")
    woT = nc.dram_tensor("woT", [DC, D], MD, kind="ExternalInput")
    bqt = nc.dram_tensor("bqt", [128, 2], F32, kind="ExternalInput")
    bkt = nc.dram_tensor("bkt", [128, 2], F32, kind="ExternalInput")
    out = nc.dram_tensor("out", [S, D], F32, kind="ExternalOutput")

    # kt-tile groups per exp op (PSUM: 2 x exp_group banks for scores
    # + 2 banks for ctx accumulation <= 8)
    groups = []
    k0 = 0
    while k0 < ST:
        g = min(exp_group, ST - k0)
        groups.append((k0, g))
        k0 += g

    lp = (nc.allow_low_precision("f32r matmul operands by design")
          if use_f32r else contextlib.nullcontext())
    with lp, tile.TileContext(nc) as tc:
        with (
            tc.tile_pool(name="persist", bufs=1) as pp,
            tc.tile_pool(name="probs", bufs=probs_bufs) as probs_pool,
            tc.tile_pool(name="norm", bufs=2) as norm_pool,
            tc.tile_pool(name="ps", bufs=sc_bufs, space="PSUM") as psp,
            tc.tile_pool(name="xtp", bufs=1) as xtp,
        ):
            # ---- persistent SBUF tensors ----
            wq = [pp.tile([128, DC], MD, tag=f"wq{k}", name=f"wq{k}") for k in range(KC)]
            wk = [pp.tile([128, DC], MD, tag=f"wk{k}", name=f"wk{k}") for k in range(KC)]
            wv = [pp.tile([128, DC], MD, tag=f"wv{k}", name=f"wv{k}") for k in range(KC)]
            wo = [pp.tile([128, D], MD, tag=f"wo{k}", name=f"wo{k}") for k in range(2)]
            qT = [pp.tile([128, S], MD, tag=f"qT{t}", name=f"qTt{t}") for t in range(2)]
            kT = [pp.tile([128, S], MD, tag=f"kT{t}", name=f"kTt{t}") for t in range(2)]
            vs = [pp.tile([128, HPC * 65], MD, tag=f"vs{s}", name=f"vs{s}") for s in range(ST)]
            ctx = [pp.tile([128, S], MD, tag=f"ctx{t}", name=f"ctxt{t}") for t in range(2)]
            bq_sb = pp.tile([128, 2], F32, tag="bq")
            bk_sb = pp.tile([128, 2], F32, tag="bk")
            ones_f32 = pp.tile([128, 128], F32, tag="ones_f32")

            for k in range(KC):
                nc.sync.dma_start(wq[k][:], wqT[k * 128:(k + 1) * 128, :])
                nc.sync.dma_start(wk[k][:], wkT[k * 128:(k + 1) * 128, :])
                nc.sync.dma_start(wv[k][:], wvT[k * 128:(k + 1) * 128, :])
            for k in range(2):
                nc.sync.dma_start(wo[k][:], woT[k * 128:(k + 1) * 128, :])
            nc.sync.dma_start(bq_sb[:], bqt[:])
            nc.sync.dma_start(bk_sb[:], bkt[:])
            nc.gpsimd.memset(ones_f32[:], 1.0)

            def proj_qk(w_tiles, xt, dst, b_sb, mt, ptag=None):
                if ptag is None:
                    ptag = "pj" if proj_own else "sc"
                for qc in range(QC):
                    ps_t = psp.tile([128, 512], F32, name="pjps",
                                    tag=ptag,
                                    bufs=(sc_bufs if ptag == "sc" else pj_bufs))
                    for k in range(KC):
                        nc.tensor.matmul(
                            ps_t[:],
                            w_tiles[k][:, mt * 128:(mt + 1) * 128],
                            xt[k][:, qc * 512:(qc + 1) * 512],
                            start=(k == 0),
                            stop=(k == KC - 1),
                        )
                    nc.vector.tensor_add(
                        dst[mt][:, qc * 512:(qc + 1) * 512],
                        ps_t[:],
                        b_sb[:, mt:mt + 1].broadcast_to([128, 512]),
                    )

            def proj_v(xt):
                for s in range(ST):
                    ps_t = psp.tile([128, DC], F32, name="vps",
                                   tag=("pj" if proj_own else "sc"),
                                   bufs=pj_bufs)
                    for k in range(KC):
                        nc.tensor.matmul(
                            ps_t[:],
                            xt[k][:, s * 128:(s + 1) * 128],
                            wv[k][:],
                            start=(k == 0),
                            stop=(k == KC - 1),
                        )
                    vv = vs[s][:].rearrange("p (h e) -> p h e", e=65)
                    nc.vector.tensor_copy(
                        vv[:, :, 0:64],
                        ps_t[:].rearrange("p (h e) -> p h e", e=64),
                    )
                    nc.vector.tensor_copy(
                        vv[:, :, 64:65],
                        ones_f32[:, None, 0:1].broadcast_to([128, HPC, 1]),
                    )

            def attention_qc(t, half, qc):
                h = 2 * t + half
                d0 = half * 64
                if True:
                    qsl = slice(qc * 512, (qc + 1) * 512)
                    cps = psp.tile([128, 512], F32, tag="ctx", name="ctxp",
                                   bufs=ctx_bufs)
                    for (k0, g) in groups:
                        sps = psp.tile([128, 512 * g], F32,
                                       tag="sc", name="scps")
                        for j in range(g):
                            kt = k0 + j
                            nc.tensor.matmul(
                                sps[:, j * 512:(j + 1) * 512],
                                kT[t][d0:d0 + 64, kt * 128:(kt + 1) * 128],
                                qT[t][d0:d0 + 64, qsl],
                                start=True,
                                stop=True,
                            )
                        pb = probs_pool.tile([128, 512 * g], MD,
                                             tag="pb", name="pb")
                        if "small_exp" in probe:
                            nc.scalar.activation(
                                pb[:, 0:64], sps[:, 0:64],
                                mybir.ActivationFunctionType.Exp)
                        else:
                            nc.scalar.activation(
                                pb[:], sps[:], mybir.ActivationFunctionType.Exp
                            )
                        for j in range(g):
                            kt = k0 + j
                            nc.tensor.matmul(
                                cps[0:65, :],
                                vs[kt][:, h * 65:h * 65 + 65],
                                pb[:, j * 512:(j + 1) * 512],
                                start=(kt == 0),
                                stop=(kt == ST - 1),
                                skip_group_check=True,
                            )
                    # normalize: ctx rows 0:64 / denom row 64
                    if "skip_norm" in probe:
                        if half == 0:
                            nc.vector.tensor_copy(ctx[t][0:64, qsl], cps[0:64, :])
                        else:
                            tmp = norm_pool.tile([128, 512], MD, tag="tmp", name="tmp")
                            nc.vector.tensor_copy(tmp[0:64, :], cps[0:64, :])
                            nc.sync.dma_start(ctx[t][64:128, qsl], tmp[0:64, :])
                        return
                    r = norm_pool.tile([128, 512], F32, tag="r", name="r")
                    nc.vector.reciprocal(r[64:65, :], cps[64:65, :])
                    r0 = norm_pool.tile([1, 512], F32, tag="r0", name="r0")
                    nc.sync.dma_start(r0[0:1, :], r[64:65, :])
                    bc = norm_pool.tile([128, 512], F32, tag="bc", name="bc")
                    nc.gpsimd.partition_broadcast(bc[0:64, :], r0[0:1, :])
                    if half == 0:
                        nc.vector.tensor_mul(
                            ctx[t][0:64, qsl], cps[0:64, :], bc[0:64, :]
                        )
                    else:
                        tmp = norm_pool.tile([128, 512], MD, tag="tmp", name="tmp")
                        nc.vector.tensor_mul(tmp[0:64, :], cps[0:64, :], bc[0:64, :])
                        # partition shift 0:64 -> 64:128 via DMA
                        nc.sync.dma_start(ctx[t][64:128, qsl], tmp[0:64, :])

            def attention(t, half):
                for qc in range(QC):
                    attention_qc(t, half, qc)

            def out_proj(qt):
                for oc in range(2):
                    po = psp.tile([128, 512], F32, name="po",
                                  tag=("pj" if proj_own else "sc"),
                                  bufs=pj_bufs)
                    for c in range(2):
                        nc.tensor.matmul(
                            po[:],
                            ctx[c][:, qt * 128:(qt + 1) * 128],
                            wo[c][:, oc * 512:(oc + 1) * 512],
                            start=(c == 0),
                            stop=(c == 1),
                        )
                    po_sb = norm_pool.tile([128, 512], F32, tag="po_sb",
                                           name="po_sb", bufs=3)
                    if po_engine == 'vector':
                        nc.vector.tensor_copy(po_sb[:], po[:])
                    else:
                        nc.scalar.copy(po_sb[:], po[:])
                    nc.sync.dma_start(
                        out[qt * 128:(qt + 1) * 128,
                            oc * 512:(oc + 1) * 512], po_sb[:]
                    )

            xt = [xtp.tile([128, S], MD, tag=f"xt{k}", name=f"xt{k}")
                  for k in range(KC)]

            def emit_body():
                for k in range(KC):
                    nc.sync.dma_start(xt[k][:], xT[k * 128:(k + 1) * 128, :])

                # heads 0,1 projections first so attention starts early
                fb = fastboot or ("fastboot" in probe)
                proj_qk(wq, xt, qT, bq_sb, 0, ptag=("sc" if fb else None))
                # fastboot=2: k-proj on the pj slots -> 4 open chains at start
                proj_qk(wk, xt, kT, bk_sb, 0,
                        ptag=(None if fastboot == 2 else ("sc" if fb else None)))
                proj_v(xt)
                attention(0, 0)
                attention(0, 1)
                # heads 2,3 projections overlap attention on heads 0,1
                proj_qk(wq, xt, qT, bq_sb, 1)
                proj_qk(wk, xt, kT, bk_sb, 1)

                attention(1, 0)
                # last head interleaved with output projection per q-chunk
                for qc in range(QC):
                    attention_qc(1, 1, qc)
                    if "no_outproj" in probe:
                        continue
                    for qt in range(4 * qc, 4 * qc + 4):
                        out_proj(qt)

            if repeat > 1:
                ET = mybir.EngineType
                with tc.For_i(0, repeat, 1, hint_engines=(
                        ET.PE, ET.Activation, ET.DVE, ET.SP, ET.Pool)):
                    emit_body()
            else:
                emit_body()

    nc.compile()
    return nc


def make_in_maps(x, Wq, bq, Wk, bk, Wv, bv, Wo, bo):
    """Host-side sharding: per-core input dict."""
    x = np.asarray(x, dtype=np.float32)
    in_maps = []
    for c in range(8):
        b, hq = divmod(c, 4)
        r0 = hq * DC
        sl = slice(r0, r0 + DC)
        in_maps.append({
            "xT": np.ascontiguousarray(x[b].T),
            "wqT": np.ascontiguousarray((np.asarray(Wq)[sl] * SCALE).T),
            "wkT": np.ascontiguousarray(np.asarray(Wk)[sl].T),
            "wvT": np.ascontiguousarray(np.asarray(Wv)[sl].T),
            "woT": np.ascontiguousarray(np.asarray(Wo)[:, sl].T),
            "bqt": np.ascontiguousarray(
                (np.asarray(bq)[sl] * SCALE).reshape(2, 128).T),
            "bkt": np.ascontiguousarray(np.asarray(bk)[sl].reshape(2, 128).T),
        })
    return [{k: np.ascontiguousarray(v, dtype=np.float32) for k, v in m.items()}
            for m in in_maps]


_NC_CACHE = {}


def _get_nc(use_f32r=True, exp_group=3, **kw):
    key = (use_f32r, exp_group, tuple(sorted(kw.items())))
    if key not in _NC_CACHE:
        _NC_CACHE[key] = build_nc(use_f32r=use_f32r, exp_group=exp_group, **kw)
    return _NC_CACHE[key]


def run(inputs, use_f32r=True, exp_group=2, proj_own=True, trace=False,
        tmpdir=None, probs_bufs=3, fastboot=1, **kw):
    """Run the SPMD kernel; returns (full_output, BassKernelResults)."""
    nc = _get_nc(use_f32r=use_f32r, exp_group=exp_group, proj_own=proj_own,
                 probs_bufs=probs_bufs, fastboot=fastboot, **kw)
    in_maps = make_in_maps(
        inputs["x"], inputs["Wq"], inputs["bq"], inputs["Wk"], inputs["bk"],
        inputs["Wv"], inputs["bv"], inputs["Wo"], inputs["bo"])
    res = run_bass_kernel_spmd(
        nc, in_maps, core_ids=list(range(8)), trace=trace, tmpdir=tmpdir)
    bo = np.asarray(inputs["bo"], dtype=np.float32)
    bv = np.asarray(inputs["bv"], dtype=np.float32)
    Wo = np.asarray(inputs["Wo"], dtype=np.float32)
    bias_vec = bo + Wo @ bv
    full = np.empty((2, S, D), dtype=np.float32)
    for b in range(2):
        acc = res.results[4 * b]["out"].astype(np.float32).copy()
        for c in range(4 * b + 1, 4 * b + 4):
            acc += res.results[c]["out"]
        full[b] = acc + bias_vec
    return full, res


def kernel(**inputs):
    full, _ = run(inputs, use_f32r=True, exp_group=2, proj_own=True,
                  probs_bufs=3, fastboot=1, trace=False)
    return full



# revision 75
# speedup vs baseline: 1.2866x; 1.2866x over previous
"""Multi-head self-attention Trainium2 kernel (Bass/Tile), 8-core SPMD.

Problem (hardcoded): B=2, S=2048, D_MODEL=1024, N_HEADS=16, HEAD_DIM=64,
mask == all-ones (no masking), dropout=0.

Sharding: core c handles batch b = c // 4 and head-quarter hq = c % 4
(heads 4*hq .. 4*hq+3).  QKV projections are column-parallel over the
head slice; attention is head-local; output projection is row-parallel
(each core produces a partial [S, D] output; host sums the 4 partials
per batch and adds bo + Wo @ bv).

v2 layout/schedule (vs the f32r baseline):
  - all matmul operands bf16 (PSUM accumulation stays f32); host converts
    x/W to bf16, halving the boot DMA bytes.
  - boot loads are single multi-descriptor mega-DMAs per tensor into SBUF
    mega-tiles, ordered by first consumption and spread over the SP /
    Activation / Pool queues (one HWDGE generation + one semaphore each).
  - emission order software-pipelines the whole program: the first
    attention unit streams its own k/v projection deps between score
    groups, (t=0) attention runs as (half0,half1) unit pairs per q-chunk
    to keep the Activation engine fed, later units each carry ~1.7us of
    projection / output-projection fill so the PE never idles, and the
    output projection trails one q-chunk behind the last head.
  - device layouts as baseline: qT/kT [128, S] per head-pair (partition =
    head dim), vs s-tiles [128, 4*65] (v + ones column -> unnormalized ctx
    + softmax denominator via one matmul), probs never normalized; ctx is
    divided by the denominator; bv folded into the host-side bias.
  - ctx_qd (default True): probs-stationary ctx accumulation — out[q, d]
    PSUM tiles (4 accumulators packed per bank, one start/stop for the
    shared 2KB zero region), denominator as a per-partition scalar, DVE
    tensor_scalar normalize, then a PE transpose writes each head half
    straight into ctx[t]'s partition range — no partition-move DMAs.
    Norms are deferred into the next unit pair so the transposes
    interleave with score groups, and within each pair the score+exp
    emission runs one group AHEAD of the ctx accumulation so backlogged
    ctx/transpose/fill work never starves the Activation engine at pair
    boundaries.  Measured (N=8193 repeat-loop slope, paired rounds):
    ctx_qd 237-274us/iter vs 305-313us for the [d, q] variant
    (ctx_qd=False).
"""

import sys

sys.path.insert(0, "/opt/trn_rl_repo")

import numpy as np
import ml_dtypes

import concourse.bacc as bacc
import concourse.tile as tile
from concourse import masks, mybir
from concourse.bass_utils import run_bass_kernel_spmd

S = 2048
D = 1024
HPC = 4          # heads per core
DH = 64          # head dim
DC = HPC * DH    # 256 = projected dims per core
KC = D // 128    # 8 contraction chunks for projections
ST = S // 128    # 16 s-tiles
QC = S // 512    # 4 q-chunks
SCALE = DH ** -0.5

F32 = mybir.dt.float32
BF16 = mybir.dt.bfloat16


def build_nc(exp_group=2, sc_bufs=2, ctx_bufs=2, pj_bufs=2, probs_bufs=4,
             norm_split=1, ctx_qd=True, repeat=1):
    """Build the SPMD Bass program (same NEFF for all 8 cores)."""
    nc = bacc.Bacc(None, target_bir_lowering=False, debug=False, num_devices=8)
    MD = BF16

    xT = nc.dram_tensor("xT", [D, S], MD, kind="ExternalInput")
    wqT = nc.dram_tensor("wqT", [D, DC], MD, kind="ExternalInput")
    wkT = nc.dram_tensor("wkT", [D, DC], MD, kind="ExternalInput")
    wvT = nc.dram_tensor("wvT", [D, DC], MD, kind="ExternalInput")
    woT = nc.dram_tensor("woT", [DC, D], MD, kind="ExternalInput")
    bqt = nc.dram_tensor("bqt", [128, 2], F32, kind="ExternalInput")
    bkt = nc.dram_tensor("bkt", [128, 2], F32, kind="ExternalInput")
    out = nc.dram_tensor("out", [S, D], F32, kind="ExternalOutput")

    # kt-tile groups per exp op
    groups = []
    k0 = 0
    while k0 < ST:
        g = min(exp_group, ST - k0)
        groups.append((k0, g))
        k0 += g

    with nc.allow_low_precision("bf16 matmul operands by design"), \
            tile.TileContext(nc) as tc:
        with (
            tc.tile_pool(name="persist", bufs=1) as pp,
            tc.tile_pool(name="probs", bufs=probs_bufs) as probs_pool,
            tc.tile_pool(name="norm", bufs=2) as norm_pool,
            tc.tile_pool(name="ps", bufs=sc_bufs, space="PSUM") as psp,
        ):
            # ---- persistent SBUF tensors ----
            # weights/x live in mega-tiles loaded by single multi-descriptor
            # DMAs (one HWDGE generation + one semaphore each); per-chunk
            # views below slice them for the matmuls.
            wqb = pp.tile([128, KC * DC], MD, tag="wqb", name="wqb")
            wkb = pp.tile([128, KC * DC], MD, tag="wkb", name="wkb")
            wvb = pp.tile([128, KC * DC], MD, tag="wvb", name="wvb")
            wob = pp.tile([128, 2 * D], MD, tag="wob", name="wob")
            xb = pp.tile([128, KC * S], MD, tag="xb", name="xb")
            wq = [wqb[:, k * DC:(k + 1) * DC] for k in range(KC)]
            wk = [wkb[:, k * DC:(k + 1) * DC] for k in range(KC)]
            wv = [wvb[:, k * DC:(k + 1) * DC] for k in range(KC)]
            wo = [wob[:, c * D:(c + 1) * D] for c in range(2)]
            xt = [xb[:, k * S:(k + 1) * S] for k in range(KC)]
            qT = [pp.tile([128, S], MD, tag=f"qT{t}", name=f"qTt{t}") for t in range(2)]
            kT = [pp.tile([128, S], MD, tag=f"kT{t}", name=f"kTt{t}") for t in range(2)]
            vs = [pp.tile([128, HPC * 65], MD, tag=f"vs{s}", name=f"vs{s}") for s in range(ST)]
            ctx = [pp.tile([128, S], MD, tag=f"ctx{t}", name=f"ctxt{t}") for t in range(2)]
            bq_sb = pp.tile([128, 2], F32, tag="bq")
            bk_sb = pp.tile([128, 2], F32, tag="bk")
            ident = (pp.tile([128, 128], MD, tag="ident", name="ident")
                     if ctx_qd else None)

            def emit_loads():
                # Mega-DMAs: one multi-descriptor transfer per tensor (one
                # HWDGE generation + one semaphore each), ordered by first
                # consumption.  x columns 0:512 (all chunks) unblock the
                # qc=0 projections; the rest of x follows in two pieces.
                xb3 = xb[:].rearrange("p (k s) -> p k s", s=S)
                xs3 = xT[:].rearrange("(k p) s -> p k s", p=128)
                wq3 = wqb[:].rearrange("p (k c) -> p k c", c=DC)
                wqs = wqT[:].rearrange("(k p) c -> p k c", p=128)
                wk3 = wkb[:].rearrange("p (k c) -> p k c", c=DC)
                wks = wkT[:].rearrange("(k p) c -> p k c", p=128)
                wv3 = wvb[:].rearrange("p (k c) -> p k c", c=DC)
                wvs = wvT[:].rearrange("(k p) c -> p k c", p=128)
                wo3 = wob[:].rearrange("p (c d) -> p c d", d=D)
                wos = woT[:].rearrange("(c p) d -> p c d", p=128)

                nc.scalar.dma_start(bq_sb[:], bqt[:])
                nc.scalar.dma_start(bk_sb[:], bkt[:])
                nc.sync.dma_start(wq3[:], wqs[:])
                nc.sync.dma_start(xb3[:, 0:4, 0:512], xs3[:, 0:4, 0:512])
                nc.sync.dma_start(xb3[:, 4:8, 0:512], xs3[:, 4:8, 0:512])
                nc.scalar.dma_start(wk3[:], wks[:])
                nc.gpsimd.dma_start(wv3[:, 0:4], wvs[:, 0:4])
                nc.gpsimd.dma_start(wv3[:, 4:8], wvs[:, 4:8])
                nc.sync.dma_start(xb3[:, :, 512:1024], xs3[:, :, 512:1024])
                nc.scalar.dma_start(xb3[:, :, 1024:2048], xs3[:, :, 1024:2048])
                nc.gpsimd.dma_start(wo3[:], wos[:])
                # softmax-denominator ones columns of the v tiles (the v
                # projection only ever writes the other columns)
                for s in range(ST):
                    vv = vs[s][:].rearrange("p (h e) -> p h e", e=65)
                    nc.gpsimd.memset(vv[:, :, 64:65], 1.0)
                if ctx_qd:
                    masks.make_identity(nc, ident[:])

            # ---- emission pieces ----
            def proj_qk_piece(w_tiles, dst, b_sb, mt, qc):
                ps_t = psp.tile([128, 512], F32, name="pjps", tag="pj",
                                bufs=pj_bufs)
                for k in range(KC):
                    nc.tensor.matmul(
                        ps_t[:],
                        w_tiles[k][:, mt * 128:(mt + 1) * 128],
                        xt[k][:, qc * 512:(qc + 1) * 512],
                        start=(k == 0),
                        stop=(k == KC - 1),
                    )
                nc.vector.tensor_add(
                    dst[mt][:, qc * 512:(qc + 1) * 512],
                    ps_t[:],
                    b_sb[:, mt:mt + 1].broadcast_to([128, 512]),
                )

            def proj_v_piece(s):
                ps_t = psp.tile([128, DC], F32, name="vps", tag="pj",
                                bufs=pj_bufs)
                for k in range(KC):
                    nc.tensor.matmul(
                        ps_t[:],
                        xt[k][:, s * 128:(s + 1) * 128],
                        wv[k][:],
                        start=(k == 0),
                        stop=(k == KC - 1),
                    )
                vv = vs[s][:].rearrange("p (h e) -> p h e", e=65)
                # drain on the Activation engine: it has slack during the
                # k/v streaming phase while the DVE drains q/k projections
                nc.scalar.copy(
                    vv[:, :, 0:64],
                    ps_t[:].rearrange("p (h e) -> p h e", e=64),
                )

            # attention state carried across group emissions: cps psum tile
            att_state = {}

            pb_state = {}

            def att_scores_exp(t, half, qc, gi):
                """Score matmuls + exp for one group of unit (t, half, qc)."""
                d0 = half * 64
                qsl = slice(qc * 512, (qc + 1) * 512)
                k0, g = groups[gi]
                sps = psp.tile([128, 512 * g], F32, tag="sc", name="scps")
                for j in range(g):
                    kt = k0 + j
                    nc.tensor.matmul(
                        sps[:, j * 512:(j + 1) * 512],
                        kT[t][d0:d0 + 64, kt * 128:(kt + 1) * 128],
                        qT[t][d0:d0 + 64, qsl],
                        start=True,
                        stop=True,
                    )
                pb = probs_pool.tile([128, 512 * g], MD, tag="pb", name="pb")
                nc.scalar.activation(
                    pb[:], sps[:], mybir.ActivationFunctionType.Exp
                )
                pb_state[(t, half, qc, gi)] = pb

            def att_ctx(t, half, qc, gi):
                """Context accumulation consuming the group's probs."""
                h = 2 * t + half
                key = (t, half, qc)
                if key not in att_state:
                    shape = [128, 4 * 65] if ctx_qd else [128, 512]
                    att_state[key] = psp.tile(shape, F32, tag="ctx",
                                              name="ctxp", bufs=ctx_bufs)
                cps = att_state[key]
                pb = pb_state.pop((t, half, qc, gi))
                k0, g = groups[gi]
                if True:
                    for j in range(g):
                        kt = k0 + j
                        if ctx_qd:
                            # probs-stationary: out[q, d] with the denominator
                            # in column 64; 65-row matmuls per 128-q block.
                            # The four accumulators share one PSUM zero
                            # region (2KB), so only the very first matmul
                            # starts (zeroing the whole region) and only the
                            # very last stops.
                            for ql in range(4):
                                nc.tensor.matmul(
                                    cps[:, ql * 65:ql * 65 + 65],
                                    pb[:, j * 512 + ql * 128:
                                       j * 512 + (ql + 1) * 128],
                                    vs[kt][:, h * 65:h * 65 + 65],
                                    start=(kt == 0 and ql == 0),
                                    stop=(kt == ST - 1 and ql == 3),
                                    skip_group_check=True,
                                )
                        else:
                            nc.tensor.matmul(
                                cps[0:65, :],
                                vs[kt][:, h * 65:h * 65 + 65],
                                pb[:, j * 512:(j + 1) * 512],
                                start=(kt == 0),
                                stop=(kt == ST - 1),
                                skip_group_check=True,
                            )

            def att_groups(t, half, qc, gset):
                for gi in gset:
                    att_scores_exp(t, half, qc, gi)
                    att_ctx(t, half, qc, gi)

            _dmaq = [nc.sync, nc.scalar, nc.gpsimd]
            _dmaq_i = [0]

            def small_dma(dst, src):
                q = _dmaq[_dmaq_i[0] % 3]
                _dmaq_i[0] += 1
                q.dma_start(dst, src)

            ctq_state = {}

            def att_norm_extract(t, half, qc):
                """v3 phase 1 (immediate, DVE-only): per-128q-block normalize
                into small [q, d] SBUF tiles, releasing the PSUM accumulator
                as early as possible."""
                cq = att_state.pop((t, half, qc))
                rq = norm_pool.tile([128, 4], F32, tag="rq", name="rq", bufs=4)
                ctqs = []
                for ql in range(4):
                    nc.vector.reciprocal(rq[:, ql:ql + 1],
                                         cq[:, ql * 65 + 64:ql * 65 + 65])
                    ctq = norm_pool.tile([128, 64], MD, tag="ctq", name="ctq",
                                         bufs=12)
                    nc.vector.tensor_scalar_mul(
                        ctq[:], cq[:, ql * 65:ql * 65 + 64], rq[:, ql:ql + 1])
                    ctqs.append(ctq)
                ctq_state[(t, half, qc)] = ctqs

            def att_norm_qd(t, half, qc, tail=False):
                """v3 phase 2 (deferrable): PE-transpose each block back to
                [d, q] directly into ctx[t]'s partition half — no
                partition-move DMAs anywhere."""
                d0 = half * 64
                ctqs = ctq_state.pop((t, half, qc))
                for ql in range(4):
                    qt = qc * 4 + ql
                    tp = psp.tile([128, 128], MD, tag="pj", name="tp",
                                  bufs=pj_bufs)
                    nc.tensor.transpose(tp[d0:d0 + 64, :], ctqs[ql][:],
                                        ident[:], tile_position=(0, d0))
                    # in the drain tail the Activation engine is idle; use
                    # it for the PSUM drains so the DVE chain isn't serial
                    if tail:
                        nc.scalar.copy(
                            ctx[t][d0:d0 + 64, qt * 128:(qt + 1) * 128],
                            tp[d0:d0 + 64, :])
                    else:
                        nc.vector.tensor_copy(
                            ctx[t][d0:d0 + 64, qt * 128:(qt + 1) * 128],
                            tp[d0:d0 + 64, :])

            def att_norm(t, half, qc, split=1, tail=False):
                """Normalize ctx rows 0:64 by denom row 64 into ctx[t].
                split>1 pipelines the chain in column sub-chunks so trailing
                consumers (tail outproj) unblock sooner."""
                if ctx_qd:
                    att_norm_qd(t, half, qc, tail=tail)
                    return
                cps = att_state.pop((t, half, qc))
                r = norm_pool.tile([128, 512], F32, tag="r", name="r")
                bc = norm_pool.tile([128, 512], F32, tag="bc", name="bc")
                w = 512 // split
                r0 = norm_pool.tile([1, 512], F32, tag="r0", name="r0")
                for i in range(split):
                    cs = slice(i * w, (i + 1) * w)
                    qsl = slice(qc * 512 + i * w, qc * 512 + (i + 1) * w)
                    nc.vector.reciprocal(r[64:65, cs], cps[64:65, cs])
                    small_dma(r0[0:1, cs], r[64:65, cs])
                    nc.gpsimd.partition_broadcast(bc[0:64, cs], r0[0:1, cs])
                    if half == 0:
                        nc.vector.tensor_mul(
                            ctx[t][0:64, qsl], cps[0:64, cs], bc[0:64, cs]
                        )
                    else:
                        tmp = norm_pool.tile([128, 512], MD, tag="tmp",
                                             name="tmp")
                        nc.vector.tensor_mul(tmp[0:64, cs], cps[0:64, cs],
                                             bc[0:64, cs])
                        # partition shift 0:64 -> 64:128 via DMA
                        small_dma(ctx[t][64:128, qsl], tmp[0:64, cs])

            po_state = {}

            def outproj_piece(qt, oc, tail=False):
                """Half an output projection column block.  In the tail the
                PSUM drain runs on the otherwise-idle Activation engine and
                each half stores separately to shorten the critical path."""
                if qt not in po_state:
                    po_state[qt] = norm_pool.tile([128, D], F32, tag="po_sb",
                                                  name="po_sb", bufs=3)
                po_sb = po_state[qt]
                po = psp.tile([128, 512], F32, name="po", tag="pj",
                              bufs=pj_bufs)
                for c in range(2):
                    nc.tensor.matmul(
                        po[:],
                        ctx[c][:, qt * 128:(qt + 1) * 128],
                        wo[c][:, oc * 512:(oc + 1) * 512],
                        start=(c == 0),
                        stop=(c == 1),
                    )
                osl = slice(oc * 512, (oc + 1) * 512)
                if tail:
                    nc.scalar.copy(po_sb[:, osl], po[:])
                    small_dma(out[qt * 128:(qt + 1) * 128, osl],
                              po_sb[:, osl])
                    if oc == 1:
                        del po_state[qt]
                else:
                    nc.vector.tensor_copy(po_sb[:, osl], po[:])
                    if oc == 1:
                        del po_state[qt]
                        small_dma(out[qt * 128:(qt + 1) * 128, :], po_sb[:])

            def emit_body():
                ng = len(groups)
                emit_loads()

                # --- preroll: first q/k/v pieces ---
                proj_qk_piece(wq, qT, bq_sb, 0, 0)
                proj_qk_piece(wk, kT, bk_sb, 0, 0)
                for s in (0, 1):
                    proj_v_piece(s)

                # --- unit pair (00,q0)+(01,q0), streaming k/v deps ---
                # group gi consumes kT cols from k-proj qc = gi//2 and
                # vs tiles 2*gi, 2*gi+1
                emitted_v = 2
                emitted_k = 1
                for gi in range(ng):
                    need_v = min(2 * (gi + 1) + 2, ST)
                    need_k = min(gi // 2 + 2, QC)
                    while emitted_k < need_k:
                        proj_qk_piece(wk, kT, bk_sb, 0, emitted_k)
                        emitted_k += 1
                    while emitted_v < need_v:
                        proj_v_piece(emitted_v)
                        emitted_v += 1
                    att_groups(0, 0, 0, [gi])
                    att_groups(0, 1, 0, [gi])
                # ctx_qd defers each pair's norms into the next pair so the
                # PE transposes interleave with score groups instead of
                # stalling the Activation engine at pair boundaries
                pending = []

                def queue_norms(t, qc):
                    if ctx_qd:
                        att_norm_extract(t, 0, qc)
                        att_norm_extract(t, 1, qc)
                        pending.append((t, 0, qc))
                        pending.append((t, 1, qc))
                    else:
                        att_norm(t, 0, qc, split=norm_split)
                        att_norm(t, 1, qc, split=norm_split)

                def flush_norm(tail=False):
                    if pending:
                        t_, h_, qc_ = pending.pop(0)
                        att_norm(t_, h_, qc_, tail=tail)
                        if t_ == 1 and h_ == 1:
                            # outproj for this q-chunk is now fully emittable
                            state["op_ready"] = 2 * (4 * qc_ + 4)

                queue_norms(0, 0)

                # --- unified fill queue for the remaining unit pairs ---
                # ordering constraints honored by construction:
                #   q(0,qc) before pair (0*,qc);  q(1,qc) before pair (1*,qc)
                #   k(1,j) before group 2j of any t=1 unit
                fills = [
                    lambda: proj_qk_piece(wq, qT, bq_sb, 0, 1),
                    lambda: proj_qk_piece(wq, qT, bq_sb, 0, 2),
                    lambda: proj_qk_piece(wq, qT, bq_sb, 0, 3),
                    lambda: proj_qk_piece(wk, kT, bk_sb, 1, 0),
                    lambda: proj_qk_piece(wk, kT, bk_sb, 1, 1),
                    lambda: proj_qk_piece(wq, qT, bq_sb, 1, 0),
                    lambda: proj_qk_piece(wk, kT, bk_sb, 1, 2),
                    lambda: proj_qk_piece(wk, kT, bk_sb, 1, 3),
                    lambda: proj_qk_piece(wq, qT, bq_sb, 1, 1),
                    lambda: proj_qk_piece(wq, qT, bq_sb, 1, 2),
                    lambda: proj_qk_piece(wq, qT, bq_sb, 1, 3),
                ]
                state = {"fi": 0, "op_done": 0, "op_ready": 0}

                def fill_slot():
                    if state["fi"] < len(fills):
                        fills[state["fi"]]()
                        state["fi"] += 1
                    elif state["op_done"] < state["op_ready"]:
                        qt, oc = divmod(state["op_done"], 2)
                        outproj_piece(qt, oc)
                        state["op_done"] += 1

                # (00,qc)+(01,qc) pairs for qc=1..3; 2 fill slots per pair.
                # scores+exp run one group AHEAD of ctx so the Activation
                # engine is never starved by backlogged ctx/transpose/fill
                # work at pair boundaries.
                for qc in range(1, QC):
                    fill_slot()   # this pair's qT chunk — must precede g0
                    att_scores_exp(0, 0, qc, 0)
                    att_scores_exp(0, 1, qc, 0)
                    for gi in range(ng):
                        if gi + 1 < ng:
                            att_scores_exp(0, 0, qc, gi + 1)
                            att_scores_exp(0, 1, qc, gi + 1)
                        att_ctx(0, 0, qc, gi)
                        att_ctx(0, 1, qc, gi)
                        if gi == 2 or gi == 5:
                            flush_norm()
                        if gi == ng // 2 - 1:
                            fill_slot()
                    queue_norms(0, qc)

                # (10,qc)+(11,qc) pairs with proj/outproj fill; outproj for
                # qt 4qc..4qc+3 unlocks once pair qc's norms are emitted
                for qc in range(QC):
                    att_scores_exp(1, 0, qc, 0)
                    att_scores_exp(1, 1, qc, 0)
                    for gi in range(ng):
                        if gi + 1 < ng:
                            att_scores_exp(1, 0, qc, gi + 1)
                            att_scores_exp(1, 1, qc, gi + 1)
                        att_ctx(1, 0, qc, gi)
                        att_ctx(1, 1, qc, gi)
                        if gi == 1 or gi == 3:
                            flush_norm()
                        if gi % 2 == 1:
                            fill_slot()
                            fill_slot()
                    if not ctx_qd:
                        last = qc == QC - 1
                        att_norm(1, 0, qc, split=2 if last else norm_split)
                        att_norm(1, 1, qc, split=2 if last else norm_split)
                        state["op_ready"] = 2 * (4 * qc + 4)
                    else:
                        att_norm_extract(1, 0, qc)
                        att_norm_extract(1, 1, qc)
                        pending.append((1, 0, qc))
                        pending.append((1, 1, qc))
                while pending:
                    flush_norm(tail=True)
                state["op_ready"] = 32
                while state["op_done"] < 32:
                    qt, oc = divmod(state["op_done"], 2)
                    outproj_piece(qt, oc, tail=True)
                    state["op_done"] += 1

            if repeat > 1:
                ET = mybir.EngineType
                with tc.For_i(0, repeat, 1, hint_engines=(
                        ET.PE, ET.Activation, ET.DVE, ET.SP, ET.Pool)):
                    emit_body()
            else:
                emit_body()

    nc.compile()
    return nc


def make_in_maps(x, Wq, bq, Wk, bk, Wv, bv, Wo, bo):
    """Host-side sharding: per-core input dict (bf16 operands)."""
    bf16 = ml_dtypes.bfloat16
    x = np.asarray(x, dtype=np.float32)
    in_maps = []
    for c in range(8):
        b, hq = divmod(c, 4)
        r0 = hq * DC
        sl = slice(r0, r0 + DC)
        in_maps.append({
            "xT": np.ascontiguousarray(x[b].T).astype(bf16),
            "wqT": np.ascontiguousarray(
                (np.asarray(Wq)[sl] * SCALE).T).astype(bf16),
            "wkT": np.ascontiguousarray(np.asarray(Wk)[sl].T).astype(bf16),
            "wvT": np.ascontiguousarray(np.asarray(Wv)[sl].T).astype(bf16),
            "woT": np.ascontiguousarray(np.asarray(Wo)[:, sl].T).astype(bf16),
            "bqt": np.ascontiguousarray(
                (np.asarray(bq)[sl] * SCALE).reshape(2, 128).T
            ).astype(np.float32),
            "bkt": np.ascontiguousarray(
                np.asarray(bk)[sl].reshape(2, 128).T).astype(np.float32),
        })
    return in_maps


_NC_CACHE = {}


def _get_nc(**kw):
    key = tuple(sorted(kw.items()))
    if key not in _NC_CACHE:
        _NC_CACHE[key] = build_nc(**kw)
    return _NC_CACHE[key]


def run(inputs, trace=False, tmpdir=None, **kw):
    """Run the SPMD kernel; returns (full_output, BassKernelResults)."""
    nc = _get_nc(**kw)
    in_maps = make_in_maps(
        inputs["x"], inputs["Wq"], inputs["bq"], inputs["Wk"], inputs["bk"],
        inputs["Wv"], inputs["bv"], inputs["Wo"], inputs["bo"])
    res = run_bass_kernel_spmd(
        nc, in_maps, core_ids=list(range(8)), trace=trace, tmpdir=tmpdir)
    bo = np.asarray(inputs["bo"], dtype=np.float32)
    bv = np.asarray(inputs["bv"], dtype=np.float32)
    Wo = np.asarray(inputs["Wo"], dtype=np.float32)
    bias_vec = bo + Wo @ bv
    full = np.empty((2, S, D), dtype=np.float32)
    for b in range(2):
        acc = res.results[4 * b]["out"].astype(np.float32).copy()
        for c in range(4 * b + 1, 4 * b + 4):
            acc += res.results[c]["out"]
        full[b] = acc + bias_vec
    return full, res


def kernel(**inputs):
    full, _ = run(inputs, trace=False)
    return full
